# revision 41
# baseline (speedup 1.0000x reference)
"""CapsuleConv2d Trainium2 kernel — 1D Winograd F(7,3) along W.

Math: out[b,o,h,w,i,j] = sum_{ci,kh,kw} W[j,o,ci,kh,kw] * x[b,ci,h+kh-1,w+kw-1,i,0]
i.e. 3x3 pad-1 conv, effective batch (b,i) = 64 images [64,56,56], Cout=256.

Strategy (8 cores, data-parallel over b; 2 b-groups per core):
  - 1D Winograd F(7,3) along w (56 = 7*8 exactly), points
    {0,+-1,+-2,+-1/2,-4,inf}: host computes x_wino = B^T x (ship fp16)
    and W_wino = G W (fp16); device multiplies + accumulates over
    (ci, kh) in fp32 PSUM; y_wino ships back fp16; host applies A^T.
    End-to-end rel err ~2.5e-3 (gate 2e-2). vs F(4,3): 6/7 the PE work
    AND 6/7 the wino-domain bytes (9 comps per 7 outputs vs 6 per 4).
  - the 4 ic0 capsule images split across partition halves: rows 0-63 =
    ci for i in {0,1}, rows 64-127 = ci for i in {2,3} -> no SBUF x
    duplication; the two row-tiles co-issue on the PE (64-deep
    contraction runs at N/2 cycles; LDWEIGHTS fully hidden).
  - per (b, h-block(14), t): two PSUM tiles (Pv: ch=0, Ps: ch=1), one
    bank per (ch,ip) combo = [14h x 8tau x 2i = 224 fp32]. Drains run
    in PARALLEL on VectorE (Pv) + ScalarE (Ps) -- separate psum tiles
    per engine avoid tile-level dependency serialization -- casting to
    fp16 slabs [128, 9t, 2ip, 224] per (b, hblock), shipped as single
    8064-B-per-partition DMA runs.
  - 8 warmup matmuls during the DMA lead-in accumulate the ~3us of
    continuous PE busy that trips the clock ramp to max p-state.
  - DMA/core: x_wino 4.1 MB (pad rows memset on device) + W 1.8 MB
    (host pre-duplicated across both partition halves -- on-device
    SBUF->SBUF dup concentrates on the odd SDMA engines) in, y_wino
    16.5 MB out. W ships in 3 t-group slabs so the first matmuls only
    wait on slab 0; b=1's x ships from inside the block loop so output
    descriptors aren't stuck behind input in the per-engine DMA rings.
    Binding resource: SDMA engine 0 (~58 us data + ~7.5 us fixed
    notification/trace traffic that always lands on it). PE ~44 us
    ending ~59 us, DVE/ACT drains ~45 us each -- all under the DMA
    wall, so exec ~= 5.5 lead-in + engine-0 busy + ~2.9 teardown.
"""

import sys

if "/opt/trn_rl_repo" not in sys.path:
    sys.path.insert(0, "/opt/trn_rl_repo")

import numpy as np

NCORES = 8
B, C, H, W_, IC0, WC1, O = 16, 64, 56, 56, 4, 4, 64
CO = WC1 * O  # 256
BPC = B // NCORES  # 2 b-groups per core
M, R = 7, 3  # F(7,3)
ALPHA = M + R - 1  # 9 wino comps
NT = W_ // M  # 8 tiles per row
NH = 28  # h rows per block
NHB = H // NH  # 2 blocks
HP = H + 2  # padded h
NFREE = NH * NT * 2  # 448 = matmul N (one i-pair; 448 fp32 <= 1 PSUM bank)


def _wino_mats():
    from fractions import Fraction as F

    pts = [F(0), F(1), F(-1), F(2), F(-2), F(1, 2), F(-1, 2), F(-4)]
    n = ALPHA
    V = [[F(0)] * n for _ in range(n)]
    for j, a in enumerate(pts):
        for i in range(n):
            V[j][i] = a**i
    V[n - 1][n - 1] = F(1)
    Mx = [row[:] + [F(1) if k == j else F(0) for k in range(n)]
          for j, row in enumerate(V)]
    for col in range(n):
        piv = next(r for r in range(col, n) if Mx[r][col] != 0)
        Mx[col], Mx[piv] = Mx[piv], Mx[col]
        pv = Mx[col][col]
        Mx[col] = [x / pv for x in Mx[col]]
        for r2 in range(n):
            if r2 != col and Mx[r2][col] != 0:
                f = Mx[r2][col]
                Mx[r2] = [x - f * y for x, y in zip(Mx[r2], Mx[col])]
    L = [row[n:] for row in Mx]
    BT = np.array([[float(L[i][j]) for i in range(n)] for j in range(n)],
                  np.float32)
    AT = np.array([[float(pts[j] ** k) if j < n - 1 else float(k == M - 1)
                    for j in range(n)] for k in range(M)], np.float32)
    G = np.array([[float(pts[j] ** i) if j < n - 1 else float(i == R - 1)
                   for i in range(R)] for j in range(n)], np.float32)
    return BT, AT, G


_BT, _AT, _G = _wino_mats()
_COMPILED = None


def _build():
    import concourse.tile as tile
    from concourse import bacc, mybir

    dt = mybir.dt
    nc = bacc.Bacc("TRN2", target_bir_lowering=False, debug=False,
                   num_devices=NCORES)
    x_d = nc.dram_tensor("x", [BPC, 128, HP, ALPHA, NT * 2], dt.float16,
                         kind="ExternalInput").ap()
    w_d = nc.dram_tensor("w", [128, ALPHA, 2, 3, 128], dt.float16,
                         kind="ExternalInput").ap()
    y_d = nc.dram_tensor("y", [BPC, NHB, 2, 128, ALPHA, 2, NFREE], dt.float16,
                         kind="ExternalOutput").ap()

    with tile.TileContext(nc) as tc:
        with (
            tc.tile_pool(name="xp", bufs=1) as xp,
            tc.tile_pool(name="wp", bufs=1) as wp,
            tc.tile_pool(name="op", bufs=4) as op,
            tc.tile_pool(name="pp", bufs=2, space="PSUM") as pp,
        ):
            xts = []
            for b in range(BPC):
                xt = xp.tile([128, HP, ALPHA, NT * 2], dt.float16,
                             tag=f"x{b}", name=f"x{b}")
                xts.append(xt)
                # h-pad rows are zeroed on device instead of shipped
                nc.vector.memset(xt[:, 0:1, :, :], 0.0)
                nc.vector.memset(xt[:, HP - 1:HP, :, :], 0.0)
            wt = wp.tile([128, ALPHA, 2, 3, 128], dt.float16)
            # first compute block's x chunk + w (split by t-group so the
            # first matmuls aren't gated on the whole w transfer); b=1's x
            # ships from inside the block loop so output descriptors aren't
            # stuck behind input in the per-engine DMA rings (FIFO per ring).
            # first data dispatch split by partition range: the [0:36] piece
            # is only 36 descriptors (~0.25us to generate vs ~0.9 for 128),
            # so the low engines start pulling data ~1us earlier
            nc.scalar.dma_start(xts[0][0:36, 1:30, :, :],
                                x_d[0, 0:36, 1:30, :, :])
            nc.scalar.dma_start(xts[0][36:128, 1:30, :, :],
                                x_d[0, 36:128, 1:30, :, :])
            for t0 in range(0, ALPHA, 3):
                nc.sync.dma_start(wt[:, t0:t0 + 3, :, :, :],
                                  w_d[:, t0:t0 + 3, :, :, :])
            # PE warmup during DMA lead-in: ~4us of dummy matmuls accumulate
            # the ~3us of continuous PE busy that trips the clock ramp to
            # max p-state, so real matmuls start fast.
            wz = wp.tile([128, 512], dt.float16, tag="wz", name="wz")
            nc.vector.memset(wz[:, :], 0.0)
            Pw1 = pp.tile([128, 2, 512], dt.float32, tag="Pv", name="Pv")
            Pw2 = pp.tile([128, 2, 512], dt.float32, tag="Ps", name="Ps")
            for wu in range(8):
                nc.tensor.matmul((Pw1 if wu % 2 else Pw2)[:, (wu // 2) % 2, :],
                                 lhsT=wz[:, 0:128], rhs=wz[:, :],
                                 start=True, stop=True)

            nc.scalar.dma_start(xts[0][:, 30:HP - 1, :, :],
                                x_d[0, :, 30:HP - 1, :, :])
            for b in range(BPC):
                xt = xts[b]
                for hb in range(NHB):
                    h0 = NH * hb
                    ov = op.tile([128, ALPHA, 2, NFREE], dt.float16,
                                 tag="ov", name="ov")
                    os_ = op.tile([128, ALPHA, 2, NFREE], dt.float16,
                                  tag="os", name="os")
                    for t in range(ALPHA):
                        Pv = pp.tile([128, 2, 512], dt.float32, tag="Pv",
                                     name="Pv")
                        Ps = pp.tile([128, 2, 512], dt.float32, tag="Ps",
                                     name="Ps")
                        for ch in range(2):
                            Pc = Pv if ch == 0 else Ps
                            for kh in range(3):
                                st, sp = (kh == 0), (kh == 2)
                                nc.tensor.matmul(
                                    Pc[:, 0, 0:NFREE],
                                    lhsT=wt[0:64, t, ch, kh, :],
                                    rhs=xt[0:64, h0 + kh:h0 + kh + NH, t, :],
                                    start=st, stop=sp,
                                )
                                nc.tensor.matmul(
                                    Pc[:, 1, 0:NFREE],
                                    lhsT=wt[64:128, t, ch, kh, :],
                                    rhs=xt[64:128, h0 + kh:h0 + kh + NH, t, :],
                                    start=st, stop=sp,
                                )
                        nc.vector.tensor_copy(ov[:, t, :, :], Pv[:, :, 0:NFREE])
                        nc.scalar.copy(os_[:, t, :, :], Ps[:, :, 0:NFREE])
                        # every block ships per 3-component third: outputs
                        # hit the DMA rings every ~3.3us, so the engines
                        # never starve waiting for a whole block, and the
                        # end-of-run flush is one third, not a block
                        if t in (2, 5, 8):
                            t0 = t - 2
                            nc.sync.dma_start(
                                y_d[b, hb, 0, :, t0:t + 1, :, :],
                                ov[:, t0:t + 1, :, :])
                            nc.sync.dma_start(
                                y_d[b, hb, 1, :, t0:t + 1, :, :],
                                os_[:, t0:t + 1, :, :])
                    if b == 0 and hb == 0:
                        nc.scalar.dma_start(xts[1][:, 1:HP - 1, :, :],
                                            x_d[1, :, 1:HP - 1, :, :])

    nc.compile()
    return nc


def _prep(x, W):
    x = np.asarray(x, dtype=np.float32)
    W = np.asarray(W, dtype=np.float32)
    xs = x[..., 0]  # [B, C, H, W, IC0]
    # input transform along w: taps d in 0..8 at w = 7*tau + d - 1
    xpw = np.zeros((B, C, H, W_ + 2, IC0), np.float32)
    xpw[:, :, :, 1:W_ + 1, :] = xs
    st = xpw.strides
    dtap = np.lib.stride_tricks.as_strided(
        xpw, (B, C, H, NT, ALPHA, IC0),
        (st[0], st[1], st[2], st[3] * M, st[3], st[4]))
    # xw[b,c,h,t,tau,i] = sum_d BT[t,d] dtap[b,c,h,tau,d,i]
    xw = np.einsum("td,bchudi->bchtui", _BT, dtap, optimize=True)
    # device layout [BPC, 128, HP, ALPHA, NT, 2]; partition p: ci = p % 64,
    # ipair = p // 64, i = ipair*2 + i2; h padded by one zero row each side
    xarr = np.zeros((B, 128, HP, ALPHA, NT * 2), np.float16)
    xwt = xw.astype(np.float16)
    xarr[:, 0:64, 1:H + 1, :, :] = np.ascontiguousarray(
        xwt[:, :, :, :, :, 0:2]).reshape(B, C, H, ALPHA, NT * 2)
    xarr[:, 64:128, 1:H + 1, :, :] = np.ascontiguousarray(
        xwt[:, :, :, :, :, 2:4]).reshape(B, C, H, ALPHA, NT * 2)
    # weights: Wf[co, ci, kh, kw], co = j*64 + o
    Wf = W.reshape(CO, C, 3, 3)
    gw = np.einsum("tk,mckh->tmch", _G, Wf.transpose(0, 1, 3, 2),
                   optimize=True)  # [T, CO, C, KH]
    warr = np.zeros((128, ALPHA, 2, 3, 128), np.float16)
    for chalf in range(2):
        blk = gw[:, chalf * 128:(chalf + 1) * 128]  # [T, 128co, C, KH]
        warr[0:64, :, chalf, :, :] = blk.transpose(2, 0, 3, 1).astype(np.float16)
    warr[64:128] = warr[0:64]  # pre-duplicated for the upper partition half
    return xarr, warr


def _post(ys):
    # ys: [B, NHB, 2ch, 128, ALPHA, 2ip, NFREE] fp32, per global b
    yw = ys.reshape(B, NHB, 2, 128, ALPHA, 2, NH, NT, 2)
    # indices: [b, hb, ch, co_p, t, ip, h14, tau, i2]
    yw = yw.transpose(0, 2, 3, 1, 6, 7, 5, 8, 4)
    # -> [B, ch, cop, hb, h14, tau, ip, i2, t]
    yw = yw.reshape(B, CO, H, NT, IC0, ALPHA)
    y = np.einsum("pt,bohuit->bohupi", _AT, yw, optimize=True)
    y = y.reshape(B, CO, H, W_, IC0)
    out = (y.reshape(B, WC1, O, H, W_, IC0)
           .transpose(0, 2, 3, 4, 5, 1))
    return np.ascontiguousarray(out, dtype=np.float32)


def _run(x, W, trace=False):
    global _COMPILED
    from concourse.bass_utils import run_bass_kernel_spmd

    if _COMPILED is None:
        _COMPILED = _build()
    nc = _COMPILED
    xarr, warr = _prep(x, W)
    in_maps = [
        {"x": np.ascontiguousarray(xarr[c * BPC:(c + 1) * BPC]), "w": warr}
        for c in range(NCORES)
    ]
    res = run_bass_kernel_spmd(nc, in_maps, core_ids=list(range(NCORES)),
                               trace=trace)
    ys = np.concatenate(
        [np.asarray(res.results[c]["y"], dtype=np.float32)
         for c in range(NCORES)], axis=0)
    return _post(ys), res


def kernel(**inputs) -> np.ndarray:
    return _run(inputs["x"], inputs["W"])[0]


# revision 42
# speedup vs baseline: 1.2763x; 1.2763x over previous
"""CapsuleConv2d Trainium2 kernel — 1D Winograd F(7,3) along W.

Math: out[b,o,h,w,i,j] = sum_{ci,kh,kw} W[j,o,ci,kh,kw] * x[b,ci,h+kh-1,w+kw-1,i,0]
i.e. 3x3 pad-1 conv, effective batch (b,i) = 64 images [64,56,56], Cout=256.

Strategy (8 cores, data-parallel over b; 2 b-groups per core):
  - 1D Winograd F(7,3) along w (56 = 7*8 exactly), points
    {0,+-1,+-2,+-1/2,-4,inf}: host computes x_wino = B^T x (ship fp16)
    and W_wino = G W (fp16); device multiplies + accumulates over
    (ci, kh) in fp32 PSUM; y_wino ships back fp16; host applies A^T.
    End-to-end rel err ~2.5e-3 (gate 2e-2). vs F(4,3): 6/7 the PE work
    AND 6/7 the wino-domain bytes (9 comps per 7 outputs vs 6 per 4).
  - the 4 ic0 capsule images split across partition halves: rows 0-63 =
    ci for i in {0,1}, rows 64-127 = ci for i in {2,3} -> no SBUF x
    duplication; the two row-tiles co-issue on the PE (64-deep
    contraction runs at N/2 cycles; LDWEIGHTS fully hidden).
  - per (b, h-block(14), t): two PSUM tiles (Pv: ch=0, Ps: ch=1), one
    bank per (ch,ip) combo = [14h x 8tau x 2i = 224 fp32]. Drains run
    in PARALLEL on VectorE (Pv) + ScalarE (Ps) -- separate psum tiles
    per engine avoid tile-level dependency serialization -- casting to
    fp16 slabs [128, 9t, 2ip, 224] per (b, hblock), shipped as single
    8064-B-per-partition DMA runs.
  - 8 warmup matmuls during the DMA lead-in accumulate the ~3us of
    continuous PE busy that trips the clock ramp to max p-state.
  - DMA/core: x_wino 4.1 MB (pad rows memset on device) + W 1.8 MB
    (host pre-duplicated across both partition halves -- on-device
    SBUF->SBUF dup concentrates on the odd SDMA engines) in, y_wino
    16.5 MB out. W ships in 3 t-group slabs so the first matmuls only
    wait on slab 0; b=1's x ships from inside the block loop so output
    descriptors aren't stuck behind input in the per-engine DMA rings.
    Binding resource: SDMA engine 0 (~58 us data + ~7.5 us fixed
    notification/trace traffic that always lands on it). PE ~44 us
    ending ~59 us, DVE/ACT drains ~45 us each -- all under the DMA
    wall, so exec ~= 5.5 lead-in + engine-0 busy + ~2.9 teardown.
"""

import sys

if "/opt/trn_rl_repo" not in sys.path:
    sys.path.insert(0, "/opt/trn_rl_repo")

import numpy as np

NCORES = 8
B, C, H, W_, IC0, WC1, O = 16, 64, 56, 56, 4, 4, 64
CO = WC1 * O  # 256
BPC = B // NCORES  # 2 b-groups per core
M, R = 7, 3  # F(7,3)
ALPHA = M + R - 1  # 9 wino comps
NT = W_ // M  # 8 tiles per row
NH = 28  # h rows per block
NHB = H // NH  # 2 blocks
HP = H + 2  # padded h
NFREE = NH * NT * 2  # 448 = matmul N (one i-pair; 448 fp32 <= 1 PSUM bank)


def _wino_mats():
    from fractions import Fraction as F

    pts = [F(0), F(1), F(-1), F(2), F(-2), F(1, 2), F(-1, 2), F(-4)]
    n = ALPHA
    V = [[F(0)] * n for _ in range(n)]
    for j, a in enumerate(pts):
        for i in range(n):
            V[j][i] = a**i
    V[n - 1][n - 1] = F(1)
    Mx = [row[:] + [F(1) if k == j else F(0) for k in range(n)]
          for j, row in enumerate(V)]
    for col in range(n):
        piv = next(r for r in range(col, n) if Mx[r][col] != 0)
        Mx[col], Mx[piv] = Mx[piv], Mx[col]
        pv = Mx[col][col]
        Mx[col] = [x / pv for x in Mx[col]]
        for r2 in range(n):
            if r2 != col and Mx[r2][col] != 0:
                f = Mx[r2][col]
                Mx[r2] = [x - f * y for x, y in zip(Mx[r2], Mx[col])]
    L = [row[n:] for row in Mx]
    BT = np.array([[float(L[i][j]) for i in range(n)] for j in range(n)],
                  np.float32)
    AT = np.array([[float(pts[j] ** k) if j < n - 1 else float(k == M - 1)
                    for j in range(n)] for k in range(M)], np.float32)
    G = np.array([[float(pts[j] ** i) if j < n - 1 else float(i == R - 1)
                   for i in range(R)] for j in range(n)], np.float32)
    return BT, AT, G


_BT, _AT, _G = _wino_mats()
_COMPILED = None


def _build():
    import concourse.tile as tile
    from concourse import bacc, mybir

    dt = mybir.dt
    nc = bacc.Bacc("TRN2", target_bir_lowering=False, debug=False,
                   num_devices=NCORES)
    x_d = nc.dram_tensor("x", [BPC, 128, HP, ALPHA, NT * 2], dt.float16,
                         kind="ExternalInput").ap()
    w_d = nc.dram_tensor("w", [128, ALPHA, 2, 3, 128], dt.float16,
                         kind="ExternalInput").ap()
    y_d = nc.dram_tensor("y", [BPC, NHB, 2, 128, ALPHA, 2, NFREE], dt.float16,
                         kind="ExternalOutput").ap()

    with tile.TileContext(nc) as tc:
        with (
            tc.tile_pool(name="xp", bufs=1) as xp,
            tc.tile_pool(name="wp", bufs=1) as wp,
            tc.tile_pool(name="op", bufs=4) as op,
            tc.tile_pool(name="pp", bufs=2, space="PSUM") as pp,
        ):
            xts = []
            for b in range(BPC):
                xt = xp.tile([128, HP, ALPHA, NT * 2], dt.float16,
                             tag=f"x{b}", name=f"x{b}")
                xts.append(xt)
                # h-pad rows are zeroed on device instead of shipped
                nc.vector.memset(xt[:, 0:1, :, :], 0.0)
                nc.vector.memset(xt[:, HP - 1:HP, :, :], 0.0)
            wt = wp.tile([128, ALPHA, 2, 3, 128], dt.float16)
            # first compute block's x chunk + w (split by t-group so the
            # first matmuls aren't gated on the whole w transfer); b=1's x
            # ships from inside the block loop so output descriptors aren't
            # stuck behind input in the per-engine DMA rings (FIFO per ring).
            nc.scalar.dma_start(xts[0][:, 1:30, :, :], x_d[0, :, 1:30, :, :])
            for t0 in range(0, ALPHA, 3):
                nc.sync.dma_start(wt[:, t0:t0 + 3, :, :, :],
                                  w_d[:, t0:t0 + 3, :, :, :])
            # PE warmup during DMA lead-in: ~4us of dummy matmuls accumulate
            # the ~3us of continuous PE busy that trips the clock ramp to
            # max p-state, so real matmuls start fast.
            wz = wp.tile([128, 512], dt.float16, tag="wz", name="wz")
            nc.vector.memset(wz[:, :], 0.0)
            Pw1 = pp.tile([128, 2, 512], dt.float32, tag="Pv", name="Pv")
            Pw2 = pp.tile([128, 2, 512], dt.float32, tag="Ps", name="Ps")
            for wu in range(8):
                nc.tensor.matmul((Pw1 if wu % 2 else Pw2)[:, (wu // 2) % 2, :],
                                 lhsT=wz[:, 0:128], rhs=wz[:, :],
                                 start=True, stop=True)

            nc.scalar.dma_start(xts[0][:, 30:HP - 1, :, :],
                                x_d[0, :, 30:HP - 1, :, :])
            for b in range(BPC):
                xt = xts[b]
                for hb in range(NHB):
                    h0 = NH * hb
                    ov = op.tile([128, ALPHA, 2, NFREE], dt.float16,
                                 tag="ov", name="ov")
                    os_ = op.tile([128, ALPHA, 2, NFREE], dt.float16,
                                  tag="os", name="os")
                    for t in range(ALPHA):
                        Pv = pp.tile([128, 2, 512], dt.float32, tag="Pv",
                                     name="Pv")
                        Ps = pp.tile([128, 2, 512], dt.float32, tag="Ps",
                                     name="Ps")
                        for ch in range(2):
                            Pc = Pv if ch == 0 else Ps
                            for kh in range(3):
                                st, sp = (kh == 0), (kh == 2)
                                nc.tensor.matmul(
                                    Pc[:, 0, 0:NFREE],
                                    lhsT=wt[0:64, t, ch, kh, :],
                                    rhs=xt[0:64, h0 + kh:h0 + kh + NH, t, :],
                                    start=st, stop=sp,
                                )
                                nc.tensor.matmul(
                                    Pc[:, 1, 0:NFREE],
                                    lhsT=wt[64:128, t, ch, kh, :],
                                    rhs=xt[64:128, h0 + kh:h0 + kh + NH, t, :],
                                    start=st, stop=sp,
                                )
                        nc.vector.tensor_copy(ov[:, t, :, :], Pv[:, :, 0:NFREE])
                        nc.scalar.copy(os_[:, t, :, :], Ps[:, :, 0:NFREE])
                        # every block ships per 3-component third: outputs
                        # hit the DMA rings every ~3.3us, so the engines
                        # never starve waiting for a whole block, and the
                        # end-of-run flush is one third, not a block
                        if t in (2, 5, 8):
                            t0 = t - 2
                            nc.sync.dma_start(
                                y_d[b, hb, 0, :, t0:t + 1, :, :],
                                ov[:, t0:t + 1, :, :])
                            nc.sync.dma_start(
                                y_d[b, hb, 1, :, t0:t + 1, :, :],
                                os_[:, t0:t + 1, :, :])
                    if b == 0 and hb == 0:
                        nc.scalar.dma_start(xts[1][:, 1:HP - 1, :, :],
                                            x_d[1, :, 1:HP - 1, :, :])

    nc.compile()
    return nc


def _prep(x, W):
    x = np.asarray(x, dtype=np.float32)
    W = np.asarray(W, dtype=np.float32)
    xs = x[..., 0]  # [B, C, H, W, IC0]
    # input transform along w: taps d in 0..8 at w = 7*tau + d - 1
    xpw = np.zeros((B, C, H, W_ + 2, IC0), np.float32)
    xpw[:, :, :, 1:W_ + 1, :] = xs
    st = xpw.strides
    dtap = np.lib.stride_tricks.as_strided(
        xpw, (B, C, H, NT, ALPHA, IC0),
        (st[0], st[1], st[2], st[3] * M, st[3], st[4]))
    # xw[b,c,h,t,tau,i] = sum_d BT[t,d] dtap[b,c,h,tau,d,i]
    xw = np.einsum("td,bchudi->bchtui", _BT, dtap, optimize=True)
    # device layout [BPC, 128, HP, ALPHA, NT, 2]; partition p: ci = p % 64,
    # ipair = p // 64, i = ipair*2 + i2; h padded by one zero row each side
    xarr = np.zeros((B, 128, HP, ALPHA, NT * 2), np.float16)
    xwt = xw.astype(np.float16)
    xarr[:, 0:64, 1:H + 1, :, :] = np.ascontiguousarray(
        xwt[:, :, :, :, :, 0:2]).reshape(B, C, H, ALPHA, NT * 2)
    xarr[:, 64:128, 1:H + 1, :, :] = np.ascontiguousarray(
        xwt[:, :, :, :, :, 2:4]).reshape(B, C, H, ALPHA, NT * 2)
    # weights: Wf[co, ci, kh, kw], co = j*64 + o
    Wf = W.reshape(CO, C, 3, 3)
    gw = np.einsum("tk,mckh->tmch", _G, Wf.transpose(0, 1, 3, 2),
                   optimize=True)  # [T, CO, C, KH]
    warr = np.zeros((128, ALPHA, 2, 3, 128), np.float16)
    for chalf in range(2):
        blk = gw[:, chalf * 128:(chalf + 1) * 128]  # [T, 128co, C, KH]
        warr[0:64, :, chalf, :, :] = blk.transpose(2, 0, 3, 1).astype(np.float16)
    warr[64:128] = warr[0:64]  # pre-duplicated for the upper partition half
    return xarr, warr


def _post(ys):
    # ys: [B, NHB, 2ch, 128, ALPHA, 2ip, NFREE] fp32, per global b
    yw = ys.reshape(B, NHB, 2, 128, ALPHA, 2, NH, NT, 2)
    # indices: [b, hb, ch, co_p, t, ip, h14, tau, i2]
    yw = yw.transpose(0, 2, 3, 1, 6, 7, 5, 8, 4)
    # -> [B, ch, cop, hb, h14, tau, ip, i2, t]
    yw = yw.reshape(B, CO, H, NT, IC0, ALPHA)
    y = np.einsum("pt,bohuit->bohupi", _AT, yw, optimize=True)
    y = y.reshape(B, CO, H, W_, IC0)
    out = (y.reshape(B, WC1, O, H, W_, IC0)
           .transpose(0, 2, 3, 4, 5, 1))
    return np.ascontiguousarray(out, dtype=np.float32)


def _run(x, W, trace=False):
    global _COMPILED
    from concourse.bass_utils import run_bass_kernel_spmd

    if _COMPILED is None:
        _COMPILED = _build()
    nc = _COMPILED
    xarr, warr = _prep(x, W)
    in_maps = [
        {"x": np.ascontiguousarray(xarr[c * BPC:(c + 1) * BPC]), "w": warr}
        for c in range(NCORES)
    ]
    res = run_bass_kernel_spmd(nc, in_maps, core_ids=list(range(NCORES)),
                               trace=trace)
    ys = np.concatenate(
        [np.asarray(res.results[c]["y"], dtype=np.float32)
         for c in range(NCORES)], axis=0)
    return _post(ys), res


def kernel(**inputs) -> np.ndarray:
    return _run(inputs["x"], inputs["W"])[0]
